# revision 5
# baseline (speedup 1.0000x reference)
"""Additive attention (Bahdanau) on 8 TRN2 NeuronCores, pure data-parallel.

Per-core shard: 8 batches. Host pre-work: transpose+cast encoder_outputs to
bf16 [b, E, S] so the device streams it with E on SBUF partitions (the layout
the proj matmul needs), plus tiny weight reshapes.

Device per batch:
  projT[h,s] = W_h.T @ encT          (PE, W_h stationary, PSUM-accum over E)
  energyT    = tanh(projT + dec@W_d) (ScalarE, add fused as per-partition bias)
  scores     = v.T @ energyT         (PE)
  p          = exp(scores)           (ScalarE; no max-subtraction: scores O(1))
  p_m        = p * mask, l = sum     (DVE fused multiply+reduce)
  w          = p_m / l               (exact zeros at masked positions)
  context    = sum_s w[s]*encT[:,s]  (DVE tensor_tensor_reduce over the SAME
                                      SBUF-resident encT tiles -> single HBM pass)
"""

import numpy as np
import ml_dtypes

B, S, E, H = 64, 2048, 1024, 512
N_CORES = 8
BPC = B // N_CORES        # batches per core
EC = E // 128             # e-chunks (8)
HC = H // 128             # h-chunks (4)
SW = 512                  # matmul moving free dim
ST = S // SW              # s-tiles (4)

BF16 = ml_dtypes.bfloat16

_cached = {}


def _build_nc():
    import concourse.bass as bass  # noqa: F401
    import concourse.tile as tile
    from concourse import bacc, mybir
    from concourse.masks import make_identity

    f32 = mybir.dt.float32
    bf16 = mybir.dt.bfloat16

    nc = bacc.Bacc("TRN2", target_bir_lowering=False, debug=False,
                   num_devices=N_CORES)

    enc_d = nc.dram_tensor("encT", [BPC, E, S], bf16, kind="ExternalInput")
    wh_d = nc.dram_tensor("w_h", [E, H], bf16, kind="ExternalInput")
    wd_d = nc.dram_tensor("w_d", [H, H], f32, kind="ExternalInput")
    decT_d = nc.dram_tensor("decT", [H, BPC], f32, kind="ExternalInput")
    v_d = nc.dram_tensor("v_col", [128, HC], bf16, kind="ExternalInput")
    mask_d = nc.dram_tensor("mask_f", [BPC, S], f32, kind="ExternalInput")
    octx_d = nc.dram_tensor("out_ctx", [BPC, E], f32, kind="ExternalOutput")
    ow_d = nc.dram_tensor("out_w", [BPC, S], f32, kind="ExternalOutput")

    with tile.TileContext(nc) as tc, \
         tc.tile_pool(name="const", bufs=1) as const_pool, \
         tc.tile_pool(name="enc", bufs=12) as enc_pool, \
         tc.tile_pool(name="energy", bufs=2) as energy_pool, \
         tc.tile_pool(name="small", bufs=2) as small_pool, \
         tc.tile_pool(name="wb", bufs=2) as wb_pool, \
         tc.tile_pool(name="psA", bufs=2, space="PSUM") as psA, \
         tc.tile_pool(name="psB", bufs=2, space="PSUM") as psB, \
         tc.tile_pool(name="psC", bufs=2, space="PSUM") as psC:

        # ---- constants / weights (loaded once) ----
        w_sb = const_pool.tile([128, EC, H], bf16, tag="w_sb")
        nc.sync.dma_start(w_sb[:], wh_d.ap().rearrange("(c p) h -> p c h", p=128))
        wd_sb = const_pool.tile([128, HC, H], f32, tag="wd_sb")
        nc.sync.dma_start(wd_sb[:], wd_d.ap().rearrange("(c p) h -> p c h", p=128))
        decT_sb = const_pool.tile([128, HC, BPC], f32, tag="decT_sb")
        nc.sync.dma_start(decT_sb[:], decT_d.ap().rearrange("(c p) b -> p c b", p=128))
        v_sb = const_pool.tile([128, HC], bf16, tag="v_sb")
        nc.sync.dma_start(v_sb[:], v_d[:, :])
        ident = const_pool.tile([128, 128], f32, tag="ident")
        make_identity(nc, ident[:])

        # ---- proj_dec = (dec @ W_d)^T  -> [128, hc, b] (bias for tanh) ----
        pdT_sb = const_pool.tile([128, HC, BPC], f32, tag="pdT_sb")
        for hc in range(HC):
            pd_ps = psC.tile([128, BPC], f32, tag="pd")
            for dc in range(HC):
                nc.tensor.matmul(
                    pd_ps[:],
                    lhsT=wd_sb[:, dc, hc * 128:(hc + 1) * 128],
                    rhs=decT_sb[:, dc, :],
                    start=(dc == 0), stop=(dc == HC - 1),
                )
            nc.scalar.copy(pdT_sb[:, hc, :], pd_ps[:])

        # ---- per batch ----
        for b in range(BPC):
            enc_t = [enc_pool.tile([128, S], bf16, tag="enc", name=f"enc_{b}_{i}")
                     for i in range(EC)]
            for ec in range(EC):
                nc.sync.dma_start(enc_t[ec][:], enc_d[b, ec * 128:(ec + 1) * 128, :])

            mask_t = small_pool.tile([1, S], f32, tag="mask")
            nc.sync.dma_start(mask_t[:], mask_d[b:b + 1, :])

            # proj + tanh -> energyT [128, hc, s] bf16
            energy = energy_pool.tile([128, HC, S], bf16, tag="energy")
            for hc in range(HC):
                for st in range(ST):
                    pj = psA.tile([128, SW], f32, tag="pj")
                    for ec in range(EC):
                        nc.tensor.matmul(
                            pj[:],
                            lhsT=w_sb[:, ec, hc * 128:(hc + 1) * 128],
                            rhs=enc_t[ec][:, st * SW:(st + 1) * SW],
                            start=(ec == 0), stop=(ec == EC - 1),
                        )
                    nc.scalar.activation(
                        energy[:, hc, st * SW:(st + 1) * SW], pj[:],
                        mybir.ActivationFunctionType.Tanh,
                        bias=pdT_sb[:, hc, b:b + 1], scale=1.0,
                    )

            # scores -> exp
            p_exp = small_pool.tile([1, S], f32, tag="p_exp")
            for st in range(ST):
                sc = psB.tile([1, SW], f32, tag="sc")
                for hc in range(HC):
                    nc.tensor.matmul(
                        sc[:],
                        lhsT=v_sb[:, hc:hc + 1],
                        rhs=energy[:, hc, st * SW:(st + 1) * SW],
                        start=(hc == 0), stop=(hc == HC - 1),
                    )
                nc.scalar.activation(
                    p_exp[:, st * SW:(st + 1) * SW], sc[:],
                    mybir.ActivationFunctionType.Exp,
                )

            # mask multiply + denominator
            p_m = small_pool.tile([1, S], f32, tag="p_m")
            lsum = small_pool.tile([1, 1], f32, tag="lsum")
            nc.vector.scalar_tensor_tensor(
                out=p_m[:], in0=p_exp[:], scalar=1.0, in1=mask_t[:],
                op0=mybir.AluOpType.bypass, op1=mybir.AluOpType.mult,
                accum_out=lsum[:],
            )
            linv = small_pool.tile([1, 1], f32, tag="linv")
            nc.vector.reciprocal(linv[:], lsum[:])

            # normalized weights: f32 out + bf16 bcast operand
            w_f32 = small_pool.tile([1, S], f32, tag="w_f32")
            nc.vector.tensor_scalar_mul(w_f32[:], p_m[:], linv[:])
            nc.sync.dma_start(ow_d[b:b + 1, :], w_f32[:])
            w_bf = small_pool.tile([1, S], bf16, tag="w_bf")
            nc.vector.tensor_scalar_mul(w_bf[:], p_m[:], linv[:])

            wb = wb_pool.tile([128, S], bf16, tag="wb")
            nc.gpsimd.partition_broadcast(wb[:], w_bf[:])

            # context: ctx[:, ec] = sum_s enc_t[ec][:, s] * wb[:, s]
            ctx = small_pool.tile([128, EC], f32, tag="ctx")
            dummy = small_pool.tile([128, 1], bf16, tag="dummy")
            for ec in range(EC):
                nc.vector.scalar_tensor_tensor(
                    out=dummy.broadcast_to((128, S)),
                    in0=enc_t[ec][:],
                    scalar=1.0,
                    in1=wb[:],
                    op0=mybir.AluOpType.bypass, op1=mybir.AluOpType.mult,
                    accum_out=ctx[:, ec:ec + 1],
                )

            # transpose [128, EC] -> [EC, 128] and store
            ctxT_ps = psC.tile([EC, 128], f32, tag="ctxT")
            nc.tensor.transpose(ctxT_ps[:], ctx[:], ident[:])
            ctxT_sb = small_pool.tile([EC, 128], f32, tag="ctxT_sb")
            nc.scalar.copy(ctxT_sb[:], ctxT_ps[:])
            nc.sync.dma_start(
                octx_d[b].rearrange("(c p) -> c p", p=128), ctxT_sb[:])

    nc.finalize()
    return nc


def _get_nc():
    if "nc" not in _cached:
        _cached["nc"] = _build_nc()
    return _cached["nc"]


def _host_prep(decoder_hidden, encoder_outputs, mask, W_h, W_d, v):
    decoder_hidden = np.asarray(decoder_hidden, dtype=np.float32)
    encoder_outputs = np.asarray(encoder_outputs)
    mask = np.asarray(mask)
    W_h = np.asarray(W_h, dtype=np.float32)
    W_d = np.asarray(W_d, dtype=np.float32)
    v = np.asarray(v, dtype=np.float32)

    wh_bf = np.ascontiguousarray(W_h.astype(BF16))
    v_col = np.ascontiguousarray(v.reshape(HC, 128).T.astype(BF16))

    from concurrent.futures import ThreadPoolExecutor

    def make_core(c):
        sl = slice(c * BPC, (c + 1) * BPC)
        encT = np.ascontiguousarray(
            encoder_outputs[sl].astype(BF16).transpose(0, 2, 1))
        return {
            "encT": encT,
            "w_h": wh_bf,
            "w_d": W_d,
            "decT": np.ascontiguousarray(decoder_hidden[sl].T),
            "v_col": v_col,
            "mask_f": mask[sl].astype(np.float32),
        }

    with ThreadPoolExecutor(max_workers=8) as ex:
        in_maps = list(ex.map(make_core, range(N_CORES)))
    return in_maps


def kernel(decoder_hidden, encoder_outputs, mask, W_h, W_d, v, _trace=False):
    from concourse.bass_utils import run_bass_kernel_spmd

    nc = _get_nc()
    in_maps = _host_prep(decoder_hidden, encoder_outputs, mask, W_h, W_d, v)
    res = run_bass_kernel_spmd(nc, in_maps, core_ids=list(range(N_CORES)),
                               trace=_trace)
    context = np.concatenate([res.results[i]["out_ctx"] for i in range(N_CORES)], axis=0)
    attn = np.concatenate([res.results[i]["out_w"] for i in range(N_CORES)], axis=0)
    if _trace:
        _cached["last_result"] = res
    return context, attn


# revision 11
# speedup vs baseline: 2.3410x; 2.3410x over previous
"""Additive attention (Bahdanau) on 8 TRN2 NeuronCores, pure data-parallel.

Per-core shard: 8 batches. Host pre-work (sharding/layout only, no math):
cast encoder_outputs to bf16, drop masked positions (they provably do not
affect the reference output: their weights are exactly 0), transpose to
[E, n_kept] per batch so the device streams it with E on SBUF partitions.
Positions are padded to a static NIDX with zeros + mask=0.

Device per batch:
  projT[h,s] = W_h.T @ encT          (PE, W_h stationary, PSUM-accum over E)
  energyT    = tanh(projT + dec@W_d) (ScalarE, add fused as per-partition bias)
  scores     = v.T @ energyT         (PE)
  p          = exp(scores)           (ScalarE; no max-subtraction: scores O(1))
  p_m        = p * mask, l = sum     (DVE fused multiply+reduce)
  context    = (sum_s p_m[s]*encT[:,s]) / l   (DVE STT-accum over the SAME
                                      SBUF-resident encT tiles -> 1 HBM pass)
Host post-work: scatter packed weights back to [B, S] (masked slots = 0).
"""

import numpy as np
import ml_dtypes

B, S, E, H = 64, 2048, 1024, 512
N_CORES = 8
BPC = B // N_CORES        # batches per core
EC = E // 128             # e-chunks (8)
HC = H // 128             # h-chunks (4)
SW = 512                  # matmul moving free dim
NIDX = 1280               # padded kept-position count (mean 1024, sigma ~23)

BF16 = ml_dtypes.bfloat16

_cached = {}


def _build_nc(nidx):
    import concourse.bass as bass  # noqa: F401
    import concourse.tile as tile
    from concourse import bacc, mybir
    from concourse.masks import make_identity

    f32 = mybir.dt.float32
    bf16 = mybir.dt.bfloat16

    s_tiles = []
    off = 0
    while off < nidx:
        w = min(SW, nidx - off)
        s_tiles.append((off, w))
        off += w

    nc = bacc.Bacc("TRN2", target_bir_lowering=False, debug=False,
                   num_devices=N_CORES)

    enc_d = nc.dram_tensor("encT", [BPC, E, nidx], bf16, kind="ExternalInput")
    wh_d = nc.dram_tensor("w_h", [E, H], bf16, kind="ExternalInput")
    wd_d = nc.dram_tensor("w_d", [H, H], f32, kind="ExternalInput")
    decT_d = nc.dram_tensor("decT", [H, BPC], f32, kind="ExternalInput")
    v_d = nc.dram_tensor("v_col", [128, HC], bf16, kind="ExternalInput")
    mask_d = nc.dram_tensor("mask_f", [BPC, nidx], f32, kind="ExternalInput")
    octx_d = nc.dram_tensor("out_ctx", [BPC, E], f32, kind="ExternalOutput")
    ow_d = nc.dram_tensor("out_w", [BPC, nidx], f32, kind="ExternalOutput")

    with tile.TileContext(nc) as tc, \
         tc.tile_pool(name="const", bufs=1) as const_pool, \
         tc.tile_pool(name="enc", bufs=2) as enc_pool, \
         tc.tile_pool(name="energy", bufs=3) as energy_pool, \
         tc.tile_pool(name="small", bufs=2) as small_pool, \
         tc.tile_pool(name="wb", bufs=2) as wb_pool, \
         tc.tile_pool(name="psA", bufs=3, space="PSUM") as psA, \
         tc.tile_pool(name="psB", bufs=2, space="PSUM") as psB, \
         tc.tile_pool(name="psC", bufs=1, space="PSUM") as psC:

        # ---- constants / weights (loaded once) ----
        w_sb = const_pool.tile([128, EC, H], bf16, tag="w_sb")
        nc.sync.dma_start(w_sb[:], wh_d.ap().rearrange("(c p) h -> p c h", p=128))
        wd_sb = const_pool.tile([128, HC, H], f32, tag="wd_sb")
        nc.sync.dma_start(wd_sb[:], wd_d.ap().rearrange("(c p) h -> p c h", p=128))
        decT_sb = const_pool.tile([128, HC, BPC], f32, tag="decT_sb")
        nc.sync.dma_start(decT_sb[:], decT_d.ap().rearrange("(c p) b -> p c b", p=128))
        v_sb = const_pool.tile([128, HC], bf16, tag="v_sb")
        nc.sync.dma_start(v_sb[:], v_d[:, :])
        ident = const_pool.tile([128, 128], f32, tag="ident")
        make_identity(nc, ident[:])

        # ---- proj_dec = (dec @ W_d)^T  -> [128, hc, b] (bias for tanh) ----
        pdT_sb = const_pool.tile([128, HC, BPC], f32, tag="pdT_sb")
        for hc in range(HC):
            pd_ps = psC.tile([128, BPC], f32, tag="pd")
            for dc in range(HC):
                nc.tensor.matmul(
                    pd_ps[:],
                    lhsT=wd_sb[:, dc, hc * 128:(hc + 1) * 128],
                    rhs=decT_sb[:, dc, :],
                    start=(dc == 0), stop=(dc == HC - 1),
                )
            nc.scalar.copy(pdT_sb[:, hc, :], pd_ps[:])

        # ---- per batch ----
        def flush_ctx(b, ctx, linv):
            # transpose [128, EC] -> [EC, 128], scale by 1/l, store.
            # Deferred into the NEXT batch's emission so the PE transpose
            # doesn't sit between batch b's scores and b+1's proj matmuls.
            linv8 = small_pool.tile([EC, 1], f32, tag="linv8",
                                    name=f"linv8_{b}")
            nc.gpsimd.partition_broadcast(linv8[:], linv[:], channels=EC)
            ctxT_ps = psC.tile([EC, 128], f32, tag="ctxT", name=f"ctxT_{b}")
            nc.tensor.transpose(ctxT_ps[:], ctx[:], ident[:])
            ctxT_sb = small_pool.tile([EC, 128], f32, tag="ctxT_sb",
                                      name=f"ctxT_sb_{b}")
            nc.scalar.activation(ctxT_sb[:], ctxT_ps[:],
                                 mybir.ActivationFunctionType.Copy,
                                 scale=linv8[:])
            nc.sync.dma_start(
                octx_d[b].rearrange("(c p) -> c p", p=128), ctxT_sb[:])

        pending = None
        for b in range(BPC):
            enc_g = enc_pool.tile([128, EC, nidx], bf16, tag="enc",
                                  name=f"enc_{b}")
            for ec in range(EC):
                nc.sync.dma_start(enc_g[:, ec, :],
                                  enc_d[b, ec * 128:(ec + 1) * 128, :])

            mask_t = small_pool.tile([1, nidx], f32, tag="mask")
            nc.sync.dma_start(mask_t[:], mask_d[b:b + 1, :])

            # proj + tanh (per s-tile) -> energyT [128, hc, sw] bf16 -> scores
            p_exp = small_pool.tile([1, nidx], f32, tag="p_exp")
            for st, (s0, sw) in enumerate(s_tiles):
                energy = energy_pool.tile([128, HC, sw], bf16, tag="energy",
                                          name=f"energy_{b}_{st}")
                for hc in range(HC):
                    pj = psA.tile([128, sw], f32, tag="pj",
                                  padded_shape=[128, SW])
                    for ec in range(EC):
                        nc.tensor.matmul(
                            pj[:],
                            lhsT=w_sb[:, ec, hc * 128:(hc + 1) * 128],
                            rhs=enc_g[:, ec, s0:s0 + sw],
                            start=(ec == 0), stop=(ec == EC - 1),
                        )
                    nc.scalar.activation(
                        energy[:, hc, :], pj[:],
                        mybir.ActivationFunctionType.Tanh,
                        bias=pdT_sb[:, hc, b:b + 1], scale=1.0,
                    )
                sc = psB.tile([1, sw], f32, tag="sc", padded_shape=[1, SW])
                for hc in range(HC):
                    nc.tensor.matmul(
                        sc[:],
                        lhsT=v_sb[:, hc:hc + 1],
                        rhs=energy[:, hc, :],
                        start=(hc == 0), stop=(hc == HC - 1),
                    )
                nc.scalar.activation(
                    p_exp[:, s0:s0 + sw], sc[:],
                    mybir.ActivationFunctionType.Exp,
                )

            # masked unnormalized weights (bf16) + denominator, one fused op
            p_mb = small_pool.tile([1, nidx], bf16, tag="p_mb")
            lsum = small_pool.tile([1, 1], f32, tag="lsum")
            nc.vector.scalar_tensor_tensor(
                out=p_mb[:], in0=p_exp[:], scalar=1.0, in1=mask_t[:],
                op0=mybir.AluOpType.bypass, op1=mybir.AluOpType.mult,
                accum_out=lsum[:],
            )
            wb = wb_pool.tile([128, nidx], bf16, tag="wb")
            nc.gpsimd.partition_broadcast(wb[:], p_mb[:])

            linv = small_pool.tile([1, 1], f32, tag="linv")
            nc.vector.reciprocal(linv[:], lsum[:])
            w_f32 = small_pool.tile([1, nidx], f32, tag="w_f32")
            nc.vector.tensor_scalar_mul(w_f32[:], p_mb[:], linv[:])
            nc.sync.dma_start(ow_d[b:b + 1, :], w_f32[:])

            # context numerator: ctx[:, ec] = sum_s enc_g[:, ec, s] * wb[:, s]
            ctx = small_pool.tile([128, EC], f32, tag="ctx")
            scr = wb_pool.tile([128, nidx], bf16, tag="scr")
            for ec in range(EC):
                nc.vector.scalar_tensor_tensor(
                    out=scr[:],
                    in0=enc_g[:, ec, :],
                    scalar=1.0,
                    in1=wb[:],
                    op0=mybir.AluOpType.bypass, op1=mybir.AluOpType.mult,
                    accum_out=ctx[:, ec:ec + 1],
                )

            if pending is not None:
                flush_ctx(*pending)
            pending = (b, ctx, linv)
        flush_ctx(*pending)

    nc.finalize()
    return nc


def _get_nc(nidx):
    key = f"nc_{nidx}"
    if key not in _cached:
        _cached[key] = _build_nc(nidx)
    return _cached[key]


def _host_prep(decoder_hidden, encoder_outputs, mask, W_h, W_d, v, nidx):
    decoder_hidden = np.asarray(decoder_hidden, dtype=np.float32)
    encoder_outputs = np.asarray(encoder_outputs)
    W_h = np.asarray(W_h, dtype=np.float32)
    W_d = np.asarray(W_d, dtype=np.float32)
    v = np.asarray(v, dtype=np.float32)

    wh_bf = np.ascontiguousarray(W_h.astype(BF16))
    v_col = np.ascontiguousarray(v.reshape(HC, 128).T.astype(BF16))

    from concurrent.futures import ThreadPoolExecutor

    def make_core(c):
        sl = slice(c * BPC, (c + 1) * BPC)
        encc = encoder_outputs[sl]
        encT = np.zeros((BPC, E, nidx), dtype=BF16)
        mask_f = np.zeros((BPC, nidx), dtype=np.float32)
        kept_c = []
        for b in range(BPC):
            kept = np.flatnonzero(mask[c * BPC + b])
            n = len(kept)
            kept_c.append(kept)
            encT[b, :, :n] = encc[b, kept].astype(BF16).T
            mask_f[b, :n] = 1.0
        return {
            "encT": encT,
            "w_h": wh_bf,
            "w_d": W_d,
            "decT": np.ascontiguousarray(decoder_hidden[sl].T),
            "v_col": v_col,
            "mask_f": mask_f,
        }, kept_c

    with ThreadPoolExecutor(max_workers=8) as ex:
        out = list(ex.map(make_core, range(N_CORES)))
    in_maps = [o[0] for o in out]
    kept_all = [k for o in out for k in o[1]]
    return in_maps, kept_all


def kernel(decoder_hidden, encoder_outputs, mask, W_h, W_d, v, _trace=False):
    from concourse.bass_utils import run_bass_kernel_spmd

    mask = np.asarray(mask)
    max_kept = int((mask != 0).sum(axis=1).max())
    nidx = NIDX if max_kept <= NIDX else S
    nc = _get_nc(nidx)
    in_maps, kept_all = _host_prep(
        decoder_hidden, encoder_outputs, mask, W_h, W_d, v, nidx)
    res = run_bass_kernel_spmd(nc, in_maps, core_ids=list(range(N_CORES)),
                               trace=_trace)
    context = np.concatenate(
        [res.results[i]["out_ctx"].reshape(BPC, E) for i in range(N_CORES)], axis=0)
    w_packed = np.concatenate(
        [res.results[i]["out_w"].reshape(BPC, nidx) for i in range(N_CORES)], axis=0)
    attn = np.zeros((B, S), dtype=np.float32)
    for b in range(B):
        kept = kept_all[b]
        attn[b, kept] = w_packed[b, :len(kept)]
    if _trace:
        _cached["last_result"] = res
    return context, attn
